# revision 12
# baseline (speedup 1.0000x reference)
"""Multi-head attention (B=8, N=1024, DIM=768, H=12) on 8 Trainium2 NeuronCores.

Sharding: pure data-parallel over the batch dimension - core c computes batch
element c end-to-end. No collectives.

v5 changes over the 225us baseline:
  - Pair-concurrent ST (QK^T) via PE row tiling: head 2t occupies PE rows
    0:64 (tile_position (0,0)), head 2t+1 rows 64:128 ((64,0)). Issued
    back-to-back, the two heads' C=64 matmuls execute concurrently in the
    array (measured ~2x for row-tiled small-K matmuls), halving ST time.
  - Pair-merged schedule: both heads of a pair run their ST+exp j-loop
    together (phase ST); their PVs + r-chains run chunk-phased as PE filler
    during the NEXT pair's ST phase. This fits PSUM: st 2x[128,1024] (4
    banks) + ot 3x[65,512] (3) + s1 [128,512] (1) = 8 banks.
  - Startup: x and pair-0 qk weights are DMA'd in k-tile chunks so the
    first projection matmuls start as soon as chunk 0 lands (x on sync
    queue, wqk on scalar, wv on sync, wp+bias on vector).
  - r-chain broadcast matmuls in bf16 (f32r moving operand measured 377ns
    vs ~215 for bf16).
  - y written to DRAM in bf16 (host casts back to fp32): halves output DMA.
  - Epilogue interleaves pair-5 PV/r-chain with reserved output-projection
    filler so the PE never idles at the stage-3 boundary (idle there
    re-throttles HAM to k=4/8 and doubled the k=5 matmul times).

Numerics: matmul inputs bf16, fp32 PSUM accumulation; softmax denominator
via ones-column of v (row 64 of the OT psum tile); reciprocal + normalize
in fp32 on DVE (denominator broadcast row in bf16).
"""

import os
import sys

for _p in ("/opt/trn_rl_repo",):
    if _p not in sys.path:
        sys.path.insert(0, _p)

import ml_dtypes
import numpy as np

import concourse.bass as bass
import concourse.tile as tile
from concourse import bacc, mybir

B, N, DIM, H = 8, 1024, 768, 12
D = DIM // H  # 64
SCALE = D ** -0.5
P = 128
KT = DIM // P        # 6 contraction tiles over dim
NT = N // P          # 8 tiles over sequence
NPAIR = H // 2       # 6 head pairs
FP = mybir.dt.float32
BF = mybir.dt.bfloat16
MMDT = BF
NP_MMDT = ml_dtypes.bfloat16


def _chunks(total, size):
    return [(lo, min(lo + size, total)) for lo in range(0, total, size)]


def build_nc():
    nc = bacc.Bacc(None, target_bir_lowering=False)
    xT = nc.dram_tensor("xT", [DIM, N], MMDT, kind="ExternalInput")
    # wqkT columns are pair-blocked: [q_t | k_t] of 128 cols each, t=0..5
    wqkT = nc.dram_tensor("wqkT", [DIM, 2 * DIM], MMDT, kind="ExternalInput")
    wvT = nc.dram_tensor("wvT", [DIM, DIM], MMDT, kind="ExternalInput")
    wpT = nc.dram_tensor("wpT", [DIM, DIM], MMDT, kind="ExternalInput")
    bias = nc.dram_tensor("bias", [1, DIM], FP, kind="ExternalInput")
    y = nc.dram_tensor("y", [N, DIM], MMDT, kind="ExternalOutput")

    with tile.TileContext(nc) as tc:
        with nc.allow_low_precision(reason="bf16 matmul inputs"):
            _body(tc, xT, wqkT, wvT, wpT, bias, y)
    nc.compile()
    return nc


def _body(tc, xT, wqkT, wvT, wpT, bias, y):
    nc = tc.nc
    Exp = mybir.ActivationFunctionType.Exp
    Mult = mybir.AluOpType.mult
    Add = mybir.AluOpType.add

    from contextlib import ExitStack
    with tc.tile_pool(name="persist", bufs=1) as persist:
      with ExitStack() as s12:
        s1w = s12.enter_context(tc.tile_pool(name="s1w", bufs=1))
        expp = s12.enter_context(tc.tile_pool(name="expp", bufs=24))
        rp = s12.enter_context(tc.tile_pool(name="rp", bufs=2))
        s1ps = s12.enter_context(tc.tile_pool(name="s1ps", bufs=1, space="PSUM"))
        stps = s12.enter_context(tc.tile_pool(name="stps", bufs=2, space="PSUM"))
        otps = s12.enter_context(tc.tile_pool(name="otps", bufs=3, space="PSUM"))

        # qkT_sb tile index 2t = q of pair t, 2t+1 = k of pair t; rows (h%2,d)
        qkT_sb = persist.tile([P, 2 * KT, N], MMDT)     # 24 KB/part
        v_sb = persist.tile([P, NT, H, D + 1], MMDT)    # 12.7 KB/part
        oT_sb = persist.tile([P, KT, N], MMDT)          # 12 KB/part
        bias_sb = persist.tile([P, DIM], FP)            # 3 KB/part
        y_acc = persist.tile([P, NT, DIM], FP)          # 24 KB/part
        ones_bf = persist.tile([1, P], MMDT)

        xT_sb = s1w.tile([P, KT, N], MMDT)              # 12 KB/part
        wqkT_sb = s1w.tile([P, KT, 2 * DIM], MMDT)      # 18 KB/part
        wvT_sb = s1w.tile([P, KT, DIM], MMDT)           # 9 KB/part
        wpT_sb = s1w.tile([P, KT, DIM], MMDT)           # 9 KB/part

        xTr = xT[:].rearrange("(t p) n -> t p n", p=P)
        wqk_t = wqkT[:].rearrange("(t p) m -> p t m", p=P)
        wv_t = wvT[:].rearrange("(t p) m -> p t m", p=P)
        wp_t = wpT[:].rearrange("(t p) m -> p t m", p=P)

        # DMA: the gating pieces first. x in per-k-tile chunks on the sync
        # queue; wqk pair-0 in 2-k-tile chunks on scalar (so the first qk
        # matmul, which needs x[k] + wqk0[k], starts as early as possible).
        for k in range(KT):
            nc.sync.dma_start(out=xT_sb[:, k], in_=xTr[k])
        for k in range(0, KT, 2):
            nc.scalar.dma_start(
                out=wqkT_sb[:, k:k + 2, 0:256], in_=wqk_t[:, k:k + 2, 0:256]
            )
        for t in range(1, NPAIR):
            nc.scalar.dma_start(
                out=wqkT_sb[:, :, t * 256:(t + 1) * 256],
                in_=wqk_t[:, :, t * 256:(t + 1) * 256],
            )
        for t in range(NPAIR):
            nc.sync.dma_start(
                out=wvT_sb[:, :, t * P:(t + 1) * P],
                in_=wv_t[:, :, t * P:(t + 1) * P],
            )
        nc.vector.memset(v_sb[:, :, :, D], 1.0)
        nc.vector.memset(ones_bf, 1.0)
        nc.scalar.dma_start(out=wpT_sb, in_=wp_t)
        nc.scalar.dma_start(out=bias_sb, in_=bias[:].to_broadcast((P, DIM)))

        # ---- stage-1 PE work generators (filler units; yields are ~us cost
        # estimates used for schedule pacing) ----
        def gen_qk(t):
            """qk pair-tile t -> qkT_sb[:, 2t] (q) and [:, 2t+1] (k)."""
            for which in range(2):
                for lo, hi in _chunks(N, 512):
                    ps = s1ps.tile([P, 512], FP, tag="s1")
                    for k in range(KT):
                        nc.tensor.matmul(
                            ps,
                            wqkT_sb[:, k, t * 256 + which * P:
                                    t * 256 + (which + 1) * P],
                            xT_sb[:, k, lo:hi],
                            start=(k == 0),
                            stop=(k == KT - 1),
                        )
                    nc.vector.tensor_copy(
                        out=qkT_sb[:, 2 * t + which, lo:hi], in_=ps)
                    yield 1.65

        def gen_v(t):
            """v pair-slice t -> v_sb[:, :, 2t:2t+2, 0:D]."""
            for half in range(2):
                ps = s1ps.tile([P, 512], FP, tag="s1")
                for jj in range(4):
                    j = half * 4 + jj
                    for k in range(KT):
                        nc.tensor.matmul(
                            ps[:, jj * P:(jj + 1) * P],
                            xT_sb[:, k, j * P:(j + 1) * P],
                            wvT_sb[:, k, t * P:(t + 1) * P],
                            start=(k == 0),
                            stop=(k == KT - 1),
                        )
                    yield 0.40
                nc.vector.tensor_copy(
                    out=v_sb[:, half * 4:(half + 1) * 4, 2 * t:2 * t + 2, 0:D],
                    in_=ps.rearrange("p (j g d) -> p j g d", g=2, d=D),
                )

        def gen_proj_partial():
            """Output-projection contributions of k-tiles 0..4, SBUF-
            accumulated into y_acc; the last units drain in the epilogue so
            the PE never idles while the final r-chains run on DVE."""
            for i in range(NT):
                for lo, hi in _chunks(DIM, 512):
                    ps = s1ps.tile([P, 512], FP, tag="s1")
                    for k in range(KT - 1):
                        nc.tensor.matmul(
                            ps[:, 0:hi - lo],
                            oT_sb[:, k, i * P:(i + 1) * P],
                            wpT_sb[:, k, lo:hi],
                            start=(k == 0),
                            stop=(k == KT - 2),
                        )
                    nc.vector.tensor_tensor(
                        out=y_acc[:, i, lo:hi], in0=ps[:, 0:hi - lo],
                        in1=bias_sb[:, lo:hi], op=Add,
                    )
                    yield 1.35 if hi - lo == 512 else 0.70

        # ---- attention ----
        ex_store = [[] for _ in range(H)]

        # Global filler work queue: [name, generator]. Pulled cost-weighted
        # so each ST j-step spans enough PE work for the previous step's
        # exps to free the st PSUM slots (else the pair's second ST
        # serializes instead of running row-tile-concurrent).
        work = []

        def pull(budget):
            while budget > 0.0 and work:
                try:
                    budget -= next(work[0][1])
                except StopIteration:
                    work.pop(0)

        def drain(name):
            while any(nm == name for nm, _ in work):
                try:
                    next(work[0][1])
                except StopIteration:
                    work.pop(0)

        def st_phase(t, budget):
            """ST+exp j-loop for BOTH heads of pair t. The two heads' C=64
            ST matmuls are issued back-to-back with explicit row-tile
            positions (0,0)/(64,0) so they execute concurrently in the PE
            array."""
            for j in range(NT):
                stA = stps.tile([P, N], FP, tag="st")
                stB = stps.tile([P, N], FP, tag="st")
                kslc = slice(j * P, (j + 1) * P)
                for lo, hi in _chunks(N, 512):
                    nc.tensor.matmul(
                        stA[:, lo:hi],
                        qkT_sb[0:D, 2 * t + 1, kslc],
                        qkT_sb[0:D, 2 * t, lo:hi],
                        start=True, stop=True,
                        tile_position=(0, 0),
                    )
                    nc.tensor.matmul(
                        stB[:, lo:hi],
                        qkT_sb[D:P, 2 * t + 1, kslc],
                        qkT_sb[D:P, 2 * t, lo:hi],
                        start=True, stop=True,
                        tile_position=(64, 0),
                    )
                exA = expp.tile([P, N], MMDT, tag="exp")
                exB = expp.tile([P, N], MMDT, tag="exp")
                nc.scalar.activation(out=exA, in_=stA, func=Exp,
                                     scale=float(SCALE))
                nc.scalar.activation(out=exB, in_=stB, func=Exp,
                                     scale=float(SCALE))
                ex_store[2 * t].append(exA)
                ex_store[2 * t + 1].append(exB)
                pull(budget)

        def pv_units(t):
            """PV + r-chain for pair t, as ~1us filler units. PV is chunk-
            phased: for each head, accumulate OT chunk c over all j (the ex
            tiles are all in SBUF by the time these run)."""
            for h in (2 * t, 2 * t + 1):
                hp = (h % 2) * D
                exs = ex_store[h]
                ota = otps.tile([D + 1, 512], FP, tag="ot")
                otb = otps.tile([D + 1, 512], FP, tag="ot")
                for c, ot in ((0, ota), (1, otb)):
                    for jhalf in range(2):
                        for j in range(jhalf * 4, jhalf * 4 + 4):
                            nc.tensor.matmul(
                                ot,
                                v_sb[:, j, h, :],
                                exs[j][:, c * 512:(c + 1) * 512],
                                start=(j == 0),
                                stop=(j == NT - 1),
                            )
                        yield 1.05
                # r-chain, PE-free: l rows -> one [1,N] SBUF staging row ->
                # fp32 approx reciprocal on the row -> partition_broadcast on
                # the (otherwise idle) GpSimd engine -> normalize fused into
                # the OT evacuation (bf16 out). No PE matmuls and no ST-pool
                # PSUM slot, so the r-chain never head-of-line-blocks the
                # tensor engine or delays the next ST pair.
                la = rp.tile([1, N], FP, tag="lrow")
                nc.vector.tensor_copy(out=la[:, 0:512], in_=ota[D:D + 1, :])
                nc.vector.tensor_copy(out=la[:, 512:N], in_=otb[D:D + 1, :])
                lr = rp.tile([1, N], FP, tag="lrec")
                nc.vector.reciprocal_approx_fast(out=lr, in_=la)
                rb_sb = rp.tile([P, N], FP, tag="rb")
                nc.gpsimd.partition_broadcast(rb_sb[0:D], lr, channels=D)
                nc.vector.tensor_tensor(
                    out=oT_sb[hp:hp + D, t, 0:512], in0=ota[0:D],
                    in1=rb_sb[0:D, 0:512], op=Mult,
                )
                nc.vector.tensor_tensor(
                    out=oT_sb[hp:hp + D, t, 512:N], in0=otb[0:D],
                    in1=rb_sb[0:D, 512:N], op=Mult,
                )
                ex_store[h] = []
                yield 0.20

        # ---- prologue: qk(0) k-outer (in the ST psum slots) so the
        # matmuls chase the per-k-tile x chunk DMAs ----
        stq = stps.tile([P, N], FP, tag="st")
        stk = stps.tile([P, N], FP, tag="st")
        for k in range(KT):
            for which, stt in ((0, stq), (1, stk)):
                for lo, hi in _chunks(N, 512):
                    nc.tensor.matmul(
                        stt[:, lo:hi],
                        wqkT_sb[:, k, which * P:(which + 1) * P],
                        xT_sb[:, k, lo:hi],
                        start=(k == 0),
                        stop=(k == KT - 1),
                    )
        nc.vector.tensor_copy(out=qkT_sb[:, 0], in_=stq)
        nc.vector.tensor_copy(out=qkT_sb[:, 1], in_=stk)

        # ---- pair pipeline ----
        work.append(["v0", gen_v(0)])
        for tt in range(1, NPAIR):
            work.append([f"qk{tt}", gen_qk(tt)])
            work.append([f"v{tt}", gen_v(tt)])
        work.append(["proj", gen_proj_partial()])

        for t in range(NPAIR):
            if t >= 1:
                work.insert(0, [f"pv{t - 1}", pv_units(t - 1)])
            st_phase(t, budget=2.8 if t == NPAIR - 1 else 2.4)
            if t + 1 < NPAIR:
                drain(f"qk{t + 1}")

        # epilogue: interleave pair-5 PV/r-chain with remaining filler so
        # the PE stays dense through the last r-chains (an idle PE here
        # re-throttles HAM and doubles the stage-3 matmul times)
        pv5 = pv_units(NPAIR - 1)
        done1 = False
        while not done1 or work:
            if not done1:
                try:
                    next(pv5)
                except StopIteration:
                    done1 = True
            pull(1.1)

      # -------- stage 3: last projection k-tile (5) + combine --------
      with (
            tc.tile_pool(name="s3y", bufs=4) as s3y,
            tc.tile_pool(name="s3ps", bufs=4, space="PSUM") as s3ps,
      ):
            yr = y[:].rearrange("(i p) e -> i p e", p=P)
            for i in range(NT):
                ps = s3ps.tile([P, DIM], FP, tag="y")
                for lo, hi in _chunks(DIM, 512):
                    nc.tensor.matmul(
                        ps[:, lo:hi],
                        oT_sb[:, KT - 1, i * P:(i + 1) * P],
                        wpT_sb[:, KT - 1, lo:hi],
                        start=True,
                        stop=True,
                    )
                y_sb = s3y.tile([P, DIM], MMDT, tag="ysb")
                nc.vector.tensor_tensor(
                    out=y_sb, in0=ps, in1=y_acc[:, i], op=Add,
                )
                nc.sync.dma_start(out=yr[i], in_=y_sb)


def prep_inputs(x, w_qkv, w_proj, b_proj):
    x = np.asarray(x, dtype=np.float32)
    w_qkv = np.asarray(w_qkv, dtype=np.float32)
    w_proj = np.asarray(w_proj, dtype=np.float32)
    b_proj = np.asarray(b_proj, dtype=np.float32)

    w_r = w_qkv.reshape(H, D, 3, DIM)  # rows ordered (h, d, qkv)
    wq = w_r[:, :, 0, :].reshape(DIM, DIM)  # rows (h, d)
    wk = w_r[:, :, 1, :].reshape(DIM, DIM)
    wv = w_r[:, :, 2, :].reshape(DIM, DIM)
    # pair-blocked qk: columns [q_t (128) | k_t (128)] for t = 0..5
    wqk_pairs = np.empty((2 * DIM, DIM), dtype=np.float32)
    for t in range(NPAIR):
        wqk_pairs[t * 256:t * 256 + P] = wq[t * P:(t + 1) * P]
        wqk_pairs[t * 256 + P:(t + 1) * 256] = wk[t * P:(t + 1) * P]
    wqkT = np.ascontiguousarray(wqk_pairs.T).astype(NP_MMDT)    # [768, 1536]
    wvT = np.ascontiguousarray(wv.T).astype(NP_MMDT)            # [768, 768]
    wpT = np.ascontiguousarray(w_proj.T).astype(NP_MMDT)        # [768, 768]
    xT = np.ascontiguousarray(x.transpose(0, 2, 1)).astype(NP_MMDT)
    bias = np.ascontiguousarray(b_proj.reshape(1, DIM))
    return xT, wqkT, wvT, wpT, bias


_NC = None
last_results = None


def get_nc():
    global _NC
    if _NC is None:
        _NC = build_nc()
    return _NC


def kernel(x, w_qkv, w_proj, b_proj):
    global last_results
    from concourse.bass_utils import run_bass_kernel_spmd

    nc = get_nc()
    xT, wqkT, wvT, wpT, bias = prep_inputs(x, w_qkv, w_proj, b_proj)
    in_maps = [
        {"xT": xT[c], "wqkT": wqkT, "wvT": wvT, "wpT": wpT, "bias": bias}
        for c in range(B)
    ]
    res = run_bass_kernel_spmd(nc, in_maps, core_ids=list(range(B)))
    last_results = res
    return np.stack(
        [np.asarray(res.results[c]["y"], dtype=np.float32) for c in range(B)],
        axis=0,
    )


# revision 26
# speedup vs baseline: 1.0720x; 1.0720x over previous
"""Multi-head attention (B=8, N=1024, DIM=768, H=12) on 8 Trainium2 NeuronCores.

Sharding: pure data-parallel over the batch dimension - core c computes batch
element c end-to-end. No collectives.

v5 changes over the 225us baseline:
  - Pair-concurrent ST (QK^T) via PE row tiling: head 2t occupies PE rows
    0:64 (tile_position (0,0)), head 2t+1 rows 64:128 ((64,0)). Issued
    back-to-back, the two heads' C=64 matmuls execute concurrently in the
    array (measured ~2x for row-tiled small-K matmuls), halving ST time.
  - Pair-merged schedule: both heads of a pair run their ST+exp j-loop
    together (phase ST); their PVs + r-chains run chunk-phased as PE filler
    during the NEXT pair's ST phase. This fits PSUM: st 2x[128,1024] (4
    banks) + ot 3x[65,512] (3) + s1 [128,512] (1) = 8 banks.
  - Startup: x and pair-0 qk weights are DMA'd in k-tile chunks so the
    first projection matmuls start as soon as chunk 0 lands (x on sync
    queue, wqk on scalar, wv on sync, wp+bias on vector).
  - r-chain broadcast matmuls in bf16 (f32r moving operand measured 377ns
    vs ~215 for bf16).
  - y written to DRAM in bf16 (host casts back to fp32): halves output DMA.
  - Epilogue interleaves pair-5 PV/r-chain with reserved output-projection
    filler so the PE never idles at the stage-3 boundary (idle there
    re-throttles HAM to k=4/8 and doubled the k=5 matmul times).

Numerics: matmul inputs bf16, fp32 PSUM accumulation; softmax denominator
via ones-column of v (row 64 of the OT psum tile); reciprocal + normalize
in fp32 on DVE (denominator broadcast row in bf16).
"""

import os
import sys

for _p in ("/opt/trn_rl_repo",):
    if _p not in sys.path:
        sys.path.insert(0, _p)

import ml_dtypes
import numpy as np

import concourse.bass as bass
import concourse.tile as tile
from concourse import bacc, mybir

B, N, DIM, H = 8, 1024, 768, 12
D = DIM // H  # 64
SCALE = D ** -0.5
P = 128
KT = DIM // P        # 6 contraction tiles over dim
NT = N // P          # 8 tiles over sequence
NPAIR = H // 2       # 6 head pairs
FP = mybir.dt.float32
BF = mybir.dt.bfloat16
MMDT = BF
NP_MMDT = ml_dtypes.bfloat16


def _chunks(total, size):
    return [(lo, min(lo + size, total)) for lo in range(0, total, size)]


def build_nc():
    nc = bacc.Bacc(None, target_bir_lowering=False)
    xT = nc.dram_tensor("xT", [DIM, N], MMDT, kind="ExternalInput")
    # wqkT columns are pair-blocked: [q_t | k_t] of 128 cols each, t=0..5
    wqkT = nc.dram_tensor("wqkT", [DIM, 2 * DIM], MMDT, kind="ExternalInput")
    wvT = nc.dram_tensor("wvT", [DIM, DIM], MMDT, kind="ExternalInput")
    wpT = nc.dram_tensor("wpT", [DIM, DIM], MMDT, kind="ExternalInput")
    bias = nc.dram_tensor("bias", [1, DIM], FP, kind="ExternalInput")
    y = nc.dram_tensor("y", [N, DIM], MMDT, kind="ExternalOutput")

    with tile.TileContext(nc) as tc:
        with nc.allow_low_precision(reason="bf16 matmul inputs"):
            _body(tc, xT, wqkT, wvT, wpT, bias, y)
    nc.compile()
    return nc


def _body(tc, xT, wqkT, wvT, wpT, bias, y):
    nc = tc.nc
    Exp = mybir.ActivationFunctionType.Exp
    Mult = mybir.AluOpType.mult
    Add = mybir.AluOpType.add

    from contextlib import ExitStack
    with tc.tile_pool(name="persist", bufs=1) as persist:
      with ExitStack() as s12:
        s1w = s12.enter_context(tc.tile_pool(name="s1w", bufs=1))
        expp = s12.enter_context(tc.tile_pool(name="expp", bufs=24))
        rp = s12.enter_context(tc.tile_pool(name="rp", bufs=2))
        s1ps = s12.enter_context(tc.tile_pool(name="s1ps", bufs=1, space="PSUM"))
        stps = s12.enter_context(tc.tile_pool(name="stps", bufs=2, space="PSUM"))
        otps = s12.enter_context(tc.tile_pool(name="otps", bufs=3, space="PSUM"))

        # qkT_sb tile index 2t = q of pair t, 2t+1 = k of pair t; rows (h%2,d)
        qkT_sb = persist.tile([P, 2 * KT, N], MMDT)     # 24 KB/part
        v_sb = persist.tile([P, NT, H, D + 1], MMDT)    # 12.7 KB/part
        oT_sb = persist.tile([P, KT, N], MMDT)          # 12 KB/part
        bias_sb = persist.tile([P, DIM], FP)            # 3 KB/part
        y_acc = persist.tile([P, NT, DIM], FP)          # 24 KB/part
        ones_row = persist.tile([1, P], MMDT)

        xT_sb = s1w.tile([P, KT, N], MMDT)              # 12 KB/part
        wqkT_sb = s1w.tile([P, KT, 2 * DIM], MMDT)      # 18 KB/part
        wvT_sb = s1w.tile([P, KT, DIM], MMDT)           # 9 KB/part
        wpT_sb = s1w.tile([P, KT, DIM], MMDT)           # 9 KB/part

        xTr = xT[:].rearrange("(t p) n -> t p n", p=P)
        wqk_t = wqkT[:].rearrange("(t p) m -> p t m", p=P)
        wv_t = wvT[:].rearrange("(t p) m -> p t m", p=P)
        wp_t = wpT[:].rearrange("(t p) m -> p t m", p=P)

        # DMA: the gating pieces first, interleaved across both HWDGE
        # queues so the k-outer qk(0) prologue can chase the chunks: even
        # x k-tiles on sync, odd x k-tiles interleaved with the wqk pair-0
        # chunks on scalar.
        nc.sync.dma_start(out=xT_sb[:, 0], in_=xTr[0])
        nc.scalar.dma_start(
            out=wqkT_sb[:, 0:2, 0:256], in_=wqk_t[:, 0:2, 0:256])
        nc.sync.dma_start(out=xT_sb[:, 2], in_=xTr[2])
        nc.scalar.dma_start(out=xT_sb[:, 1], in_=xTr[1])
        nc.sync.dma_start(out=xT_sb[:, 4], in_=xTr[4])
        nc.scalar.dma_start(
            out=wqkT_sb[:, 2:4, 0:256], in_=wqk_t[:, 2:4, 0:256])
        nc.scalar.dma_start(out=xT_sb[:, 3], in_=xTr[3])
        nc.scalar.dma_start(
            out=wqkT_sb[:, 4:6, 0:256], in_=wqk_t[:, 4:6, 0:256])
        nc.scalar.dma_start(out=xT_sb[:, 5], in_=xTr[5])
        for t in range(1, NPAIR):
            nc.scalar.dma_start(
                out=wqkT_sb[:, :, t * 256:(t + 1) * 256],
                in_=wqk_t[:, :, t * 256:(t + 1) * 256],
            )
        for t in range(NPAIR):
            nc.sync.dma_start(
                out=wvT_sb[:, :, t * P:(t + 1) * P],
                in_=wv_t[:, :, t * P:(t + 1) * P],
            )
        nc.vector.memset(v_sb[:, :, :, D], 1.0)
        nc.vector.memset(ones_row, 1.0)
        nc.scalar.dma_start(out=wpT_sb, in_=wp_t)
        nc.sync.dma_start(out=bias_sb, in_=bias[:].to_broadcast((P, DIM)))
        # warm the ScalarE activation table for Exp during the DMA-in phase
        # (the implicit ACT_TABLE_LOAD is 1.3us and otherwise lands in front
        # of the first real exp)
        warm = rp.tile([1, 16], FP, tag="warm")
        nc.scalar.activation(out=warm, in_=ones_row[:, 0:16],
                             func=Exp, scale=1.0)

        # ---- stage-1 PE work generators (filler units; yields are ~us cost
        # estimates used for schedule pacing) ----
        def gen_qk(t):
            """qk pair-tile t -> qkT_sb[:, 2t] (q) and [:, 2t+1] (k)."""
            for which in range(2):
                for lo, hi in _chunks(N, 512):
                    ps = s1ps.tile([P, 512], FP, tag="s1")
                    for k in range(KT):
                        nc.tensor.matmul(
                            ps,
                            wqkT_sb[:, k, t * 256 + which * P:
                                    t * 256 + (which + 1) * P],
                            xT_sb[:, k, lo:hi],
                            start=(k == 0),
                            stop=(k == KT - 1),
                        )
                    nc.vector.tensor_copy(
                        out=qkT_sb[:, 2 * t + which, lo:hi], in_=ps)
                    yield 1.65

        def gen_v(t):
            """v pair-slice t -> v_sb[:, :, 2t:2t+2, 0:D]."""
            for half in range(2):
                ps = s1ps.tile([P, 512], FP, tag="s1")
                for jj in range(4):
                    j = half * 4 + jj
                    for k in range(KT):
                        nc.tensor.matmul(
                            ps[:, jj * P:(jj + 1) * P],
                            xT_sb[:, k, j * P:(j + 1) * P],
                            wvT_sb[:, k, t * P:(t + 1) * P],
                            start=(k == 0),
                            stop=(k == KT - 1),
                        )
                    yield 0.40
                nc.vector.tensor_copy(
                    out=v_sb[:, half * 4:(half + 1) * 4, 2 * t:2 * t + 2, 0:D],
                    in_=ps.rearrange("p (j g d) -> p j g d", g=2, d=D),
                )

        def gen_proj_partial():
            """Output-projection contributions of k-tiles 0..4, SBUF-
            accumulated into y_acc; the last units drain in the epilogue so
            the PE never idles while the final r-chains run on DVE."""
            for i in range(NT):
                for lo, hi in _chunks(DIM, 512):
                    ps = s1ps.tile([P, 512], FP, tag="s1")
                    for k in range(KT - 1):
                        nc.tensor.matmul(
                            ps[:, 0:hi - lo],
                            oT_sb[:, k, i * P:(i + 1) * P],
                            wpT_sb[:, k, lo:hi],
                            start=(k == 0),
                            stop=(k == KT - 2),
                        )
                    nc.vector.tensor_tensor(
                        out=y_acc[:, i, lo:hi], in0=ps[:, 0:hi - lo],
                        in1=bias_sb[:, lo:hi], op=Add,
                    )
                    yield 1.35 if hi - lo == 512 else 0.70

        # ---- attention ----
        ex_store = [[] for _ in range(H)]

        # Global filler work queue: [name, generator]. Pulled cost-weighted
        # so each ST j-step spans enough PE work for the previous step's
        # exps to free the st PSUM slots (else the pair's second ST
        # serializes instead of running row-tile-concurrent).
        work = []

        def pull(budget):
            while budget > 0.0 and work:
                try:
                    budget -= next(work[0][1])
                except StopIteration:
                    work.pop(0)

        def drain(name):
            while any(nm == name for nm, _ in work):
                try:
                    next(work[0][1])
                except StopIteration:
                    work.pop(0)

        # Deferred work cells. pend_exp holds the last-issued ST pair whose
        # exps go out at the TOP of the next cycle (so ST matmuls are issued
        # ~a full cycle after the exps that free their PSUM slots, and never
        # head-of-line-block the PE). pend_rc holds an r-chain tail closure
        # (lbc matmuls + reciprocal + normalize) flushed one unit after its
        # DVE prefix was issued, so the PE reaches the lbc matmuls after the
        # prefix has had time to run.
        pend_exp = [None]
        pend_rc = []

        def flush_exps():
            if pend_exp[0] is None:
                return
            stA, stB, hA, hB = pend_exp[0]
            pend_exp[0] = None
            exA = expp.tile([P, N], MMDT, tag="exp")
            exB = expp.tile([P, N], MMDT, tag="exp")
            nc.scalar.activation(out=exA, in_=stA, func=Exp,
                                 scale=float(SCALE))
            nc.scalar.activation(out=exB, in_=stB, func=Exp,
                                 scale=float(SCALE))
            ex_store[hA].append(exA)
            ex_store[hB].append(exB)

        def flush_rc():
            if pend_rc:
                pend_rc.pop(0)()

        def st_phase(t, budget):
            """ST+exp j-loop for BOTH heads of pair t. The two heads' C=64
            ST matmuls are issued back-to-back with explicit row-tile
            positions (0,0)/(64,0) so they execute concurrently in the PE
            array."""
            for j in range(NT):
                flush_exps()
                pull(budget)
                flush_rc()
                stA = stps.tile([P, N], FP, tag="st")
                stB = stps.tile([P, N], FP, tag="st")
                kslc = slice(j * P, (j + 1) * P)
                for lo, hi in _chunks(N, 512):
                    nc.tensor.matmul(
                        stA[:, lo:hi],
                        qkT_sb[0:D, 2 * t + 1, kslc],
                        qkT_sb[0:D, 2 * t, lo:hi],
                        start=True, stop=True,
                        tile_position=(0, 0),
                    )
                    nc.tensor.matmul(
                        stB[:, lo:hi],
                        qkT_sb[D:P, 2 * t + 1, kslc],
                        qkT_sb[D:P, 2 * t, lo:hi],
                        start=True, stop=True,
                        tile_position=(64, 0),
                    )
                pend_exp[0] = (stA, stB, 2 * t, 2 * t + 1)

        def pv_units(t):
            """PV + r-chain for pair t, as ~1us filler units. PV is chunk-
            phased: for each head, accumulate OT chunk c over all j (the ex
            tiles are all in SBUF by the time these run). Each unit first
            flushes any deferred r-chain tail AFTER issuing its own PE work,
            giving the tail's DVE prefix (the l-row copies) time to run."""
            for h in (2 * t, 2 * t + 1):
                hp = (h % 2) * D
                exs = ex_store[h]
                ota = otps.tile([P, 512], FP, tag="ot")
                otb = otps.tile([P, 512], FP, tag="ot")
                for c, ot in ((0, ota), (1, otb)):
                    for jhalf in range(2):
                        for j in range(jhalf * 4, jhalf * 4 + 4):
                            nc.tensor.matmul(
                                ot[0:D + 1, :],
                                v_sb[:, j, h, :],
                                exs[j][:, c * 512:(c + 1) * 512],
                                start=(j == 0),
                                stop=(j == NT - 1),
                            )
                        flush_rc()
                        yield 1.05
                # r-chain: l rows (bf16) to SBUF now (DVE prefix); the two
                # tail stages (per chunk: ones x l broadcast matmul into an
                # s1-pool PSUM slot -> fp32 approx reciprocal -> normalize
                # fused into the OT evacuation) are deferred to later flush
                # points so the PE reaches each broadcast matmul after its
                # DVE dependencies completed.
                la = rp.tile([1, 512], MMDT, tag="lrowa")
                lb = rp.tile([1, 512], MMDT, tag="lrowb")
                nc.vector.tensor_copy(out=la, in_=ota[D:D + 1, :])
                nc.vector.tensor_copy(out=lb, in_=otb[D:D + 1, :])

                def rc_stage(lrow, ot, lo, hi, hp=hp):
                    def run():
                        lbc = s1ps.tile([P, 512], FP, tag="s1")
                        nc.tensor.matmul(lbc, ones_row, lrow,
                                         start=True, stop=True)
                        rb_sb = rp.tile([P, 512], FP, tag="rb")
                        nc.vector.reciprocal_approx_fast(out=rb_sb, in_=lbc)
                        nc.vector.tensor_tensor(
                            out=oT_sb[hp:hp + D, t, lo:hi], in0=ot[0:D],
                            in1=rb_sb[0:D], op=Mult,
                        )
                    return run

                pend_rc.append(rc_stage(la, ota, 0, 512))
                pend_rc.append(rc_stage(lb, otb, 512, N))
                ex_store[h] = []
                yield 0.40

        # ---- prologue: qk(0) k-outer (in the ST psum slots) so the
        # matmuls chase the per-k-tile x chunk DMAs ----
        stq = stps.tile([P, N], FP, tag="st")
        stk = stps.tile([P, N], FP, tag="st")
        for k in range(KT):
            for which, stt in ((0, stq), (1, stk)):
                for lo, hi in _chunks(N, 512):
                    nc.tensor.matmul(
                        stt[:, lo:hi],
                        wqkT_sb[:, k, which * P:(which + 1) * P],
                        xT_sb[:, k, lo:hi],
                        start=(k == 0),
                        stop=(k == KT - 1),
                    )
        nc.vector.tensor_copy(out=qkT_sb[:, 0], in_=stq)
        nc.vector.tensor_copy(out=qkT_sb[:, 1], in_=stk)

        # ---- pair pipeline ----
        work.append(["v0", gen_v(0)])
        for tt in range(1, NPAIR):
            work.append([f"qk{tt}", gen_qk(tt)])
            work.append([f"v{tt}", gen_v(tt)])
        work.append(["proj", gen_proj_partial()])

        for t in range(NPAIR):
            if t >= 1:
                work.insert(0, [f"pv{t - 1}", pv_units(t - 1)])
            st_phase(t, budget=3.0 if t == NPAIR - 1 else 2.55)
            if t + 1 < NPAIR:
                drain(f"qk{t + 1}")

        # epilogue: interleave pair-5 PV/r-chain with remaining filler so
        # the PE stays dense through the last r-chains (an idle PE here
        # re-throttles HAM and doubles the stage-3 matmul times)
        flush_exps()
        pv5 = pv_units(NPAIR - 1)
        for _ in pv5:
            pull(0.55)
            flush_rc()
        # bridge the final r-chain DVE latency with the remaining proj
        # filler so the PE stays busy until stage 3 can start
        while pend_rc or work:
            flush_rc()
            pull(0.8)

      # -------- stage 3: last projection k-tile (5) + combine --------
      with (
            tc.tile_pool(name="s3y", bufs=4) as s3y,
            tc.tile_pool(name="s3ps", bufs=4, space="PSUM") as s3ps,
      ):
            yr = y[:].rearrange("(i p) e -> i p e", p=P)
            for i in range(NT):
                ps = s3ps.tile([P, DIM], FP, tag="y")
                for lo, hi in _chunks(DIM, 512):
                    nc.tensor.matmul(
                        ps[:, lo:hi],
                        oT_sb[:, KT - 1, i * P:(i + 1) * P],
                        wpT_sb[:, KT - 1, lo:hi],
                        start=True,
                        stop=True,
                    )
                y_sb = s3y.tile([P, DIM], MMDT, tag="ysb")
                nc.vector.tensor_tensor(
                    out=y_sb, in0=ps, in1=y_acc[:, i], op=Add,
                )
                nc.sync.dma_start(out=yr[i], in_=y_sb)


def prep_inputs(x, w_qkv, w_proj, b_proj):
    x = np.asarray(x, dtype=np.float32)
    w_qkv = np.asarray(w_qkv, dtype=np.float32)
    w_proj = np.asarray(w_proj, dtype=np.float32)
    b_proj = np.asarray(b_proj, dtype=np.float32)

    w_r = w_qkv.reshape(H, D, 3, DIM)  # rows ordered (h, d, qkv)
    wq = w_r[:, :, 0, :].reshape(DIM, DIM)  # rows (h, d)
    wk = w_r[:, :, 1, :].reshape(DIM, DIM)
    wv = w_r[:, :, 2, :].reshape(DIM, DIM)
    # pair-blocked qk: columns [q_t (128) | k_t (128)] for t = 0..5
    wqk_pairs = np.empty((2 * DIM, DIM), dtype=np.float32)
    for t in range(NPAIR):
        wqk_pairs[t * 256:t * 256 + P] = wq[t * P:(t + 1) * P]
        wqk_pairs[t * 256 + P:(t + 1) * 256] = wk[t * P:(t + 1) * P]
    wqkT = np.ascontiguousarray(wqk_pairs.T).astype(NP_MMDT)    # [768, 1536]
    wvT = np.ascontiguousarray(wv.T).astype(NP_MMDT)            # [768, 768]
    wpT = np.ascontiguousarray(w_proj.T).astype(NP_MMDT)        # [768, 768]
    xT = np.ascontiguousarray(x.transpose(0, 2, 1)).astype(NP_MMDT)
    bias = np.ascontiguousarray(b_proj.reshape(1, DIM))
    return xT, wqkT, wvT, wpT, bias


_NC = None
last_results = None


def get_nc():
    global _NC
    if _NC is None:
        _NC = build_nc()
    return _NC


def kernel(x, w_qkv, w_proj, b_proj):
    global last_results
    from concourse.bass_utils import run_bass_kernel_spmd

    nc = get_nc()
    xT, wqkT, wvT, wpT, bias = prep_inputs(x, w_qkv, w_proj, b_proj)
    in_maps = [
        {"xT": xT[c], "wqkT": wqkT, "wvT": wvT, "wpT": wpT, "bias": bias}
        for c in range(B)
    ]
    res = run_bass_kernel_spmd(nc, in_maps, core_ids=list(range(B)))
    last_results = res
    return np.stack(
        [np.asarray(res.results[c]["y"], dtype=np.float32) for c in range(B)],
        axis=0,
    )


# revision 27
# speedup vs baseline: 1.1148x; 1.0399x over previous
"""Multi-head attention (B=8, N=1024, DIM=768, H=12) on 8 Trainium2 NeuronCores.

Sharding: pure data-parallel over the batch dimension - core c computes batch
element c end-to-end. No collectives.

v5 changes over the 225us baseline:
  - Pair-concurrent ST (QK^T) via PE row tiling: head 2t occupies PE rows
    0:64 (tile_position (0,0)), head 2t+1 rows 64:128 ((64,0)). Issued
    back-to-back, the two heads' C=64 matmuls execute concurrently in the
    array (measured ~2x for row-tiled small-K matmuls), halving ST time.
  - Pair-merged schedule: both heads of a pair run their ST+exp j-loop
    together (phase ST); their PVs + r-chains run chunk-phased as PE filler
    during the NEXT pair's ST phase. This fits PSUM: st 2x[128,1024] (4
    banks) + ot 3x[65,512] (3) + s1 [128,512] (1) = 8 banks.
  - Startup: x and pair-0 qk weights are DMA'd in k-tile chunks so the
    first projection matmuls start as soon as chunk 0 lands (x on sync
    queue, wqk on scalar, wv on sync, wp+bias on vector).
  - r-chain broadcast matmuls in bf16 (f32r moving operand measured 377ns
    vs ~215 for bf16).
  - y written to DRAM in bf16 (host casts back to fp32): halves output DMA.
  - Epilogue interleaves pair-5 PV/r-chain with reserved output-projection
    filler so the PE never idles at the stage-3 boundary (idle there
    re-throttles HAM to k=4/8 and doubled the k=5 matmul times).

Numerics: matmul inputs bf16, fp32 PSUM accumulation; softmax denominator
via ones-column of v (row 64 of the OT psum tile); reciprocal + normalize
in fp32 on DVE (denominator broadcast row in bf16).
"""

import os
import sys

for _p in ("/opt/trn_rl_repo",):
    if _p not in sys.path:
        sys.path.insert(0, _p)

import ml_dtypes
import numpy as np

import concourse.bass as bass
import concourse.tile as tile
from concourse import bacc, mybir

B, N, DIM, H = 8, 1024, 768, 12
D = DIM // H  # 64
SCALE = D ** -0.5
P = 128
KT = DIM // P        # 6 contraction tiles over dim
NT = N // P          # 8 tiles over sequence
NPAIR = H // 2       # 6 head pairs
FP = mybir.dt.float32
BF = mybir.dt.bfloat16
MMDT = BF
NP_MMDT = ml_dtypes.bfloat16


def _chunks(total, size):
    return [(lo, min(lo + size, total)) for lo in range(0, total, size)]


def build_nc():
    nc = bacc.Bacc(None, target_bir_lowering=False)
    xT = nc.dram_tensor("xT", [DIM, N], MMDT, kind="ExternalInput")
    # wqkT columns are pair-blocked: [q_t | k_t] of 128 cols each, t=0..5
    wqkT = nc.dram_tensor("wqkT", [DIM, 2 * DIM], MMDT, kind="ExternalInput")
    wvT = nc.dram_tensor("wvT", [DIM, DIM], MMDT, kind="ExternalInput")
    wpT = nc.dram_tensor("wpT", [DIM, DIM], MMDT, kind="ExternalInput")
    bias = nc.dram_tensor("bias", [1, DIM], FP, kind="ExternalInput")
    y = nc.dram_tensor("y", [N, DIM], MMDT, kind="ExternalOutput")

    with tile.TileContext(nc) as tc:
        with nc.allow_low_precision(reason="bf16 matmul inputs"):
            _body(tc, xT, wqkT, wvT, wpT, bias, y)
    nc.compile()
    return nc


def _body(tc, xT, wqkT, wvT, wpT, bias, y):
    nc = tc.nc
    Exp = mybir.ActivationFunctionType.Exp
    Mult = mybir.AluOpType.mult
    Add = mybir.AluOpType.add

    from contextlib import ExitStack
    with tc.tile_pool(name="persist", bufs=1) as persist:
      with ExitStack() as s12:
        s1w = s12.enter_context(tc.tile_pool(name="s1w", bufs=1))
        expp = s12.enter_context(tc.tile_pool(name="expp", bufs=24))
        rp = s12.enter_context(tc.tile_pool(name="rp", bufs=2))
        s1ps = s12.enter_context(tc.tile_pool(name="s1ps", bufs=1, space="PSUM"))
        stps = s12.enter_context(tc.tile_pool(name="stps", bufs=2, space="PSUM"))
        otps = s12.enter_context(tc.tile_pool(name="otps", bufs=3, space="PSUM"))

        # qkT_sb tile index 2t = q of pair t, 2t+1 = k of pair t; rows (h%2,d)
        qkT_sb = persist.tile([P, 2 * KT, N], MMDT)     # 24 KB/part
        v_sb = persist.tile([P, NT, H, D + 1], MMDT)    # 12.7 KB/part
        oT_sb = persist.tile([P, KT, N], MMDT)          # 12 KB/part
        bias_sb = persist.tile([P, DIM], FP)            # 3 KB/part
        y_acc = persist.tile([P, NT, DIM], FP)          # 24 KB/part
        ones_row = persist.tile([1, P], MMDT)

        xT_sb = s1w.tile([P, KT, N], MMDT)              # 12 KB/part
        wqkT_sb = s1w.tile([P, KT, 2 * DIM], MMDT)      # 18 KB/part
        wvT_sb = s1w.tile([P, KT, DIM], MMDT)           # 9 KB/part
        wpT_sb = s1w.tile([P, KT, DIM], MMDT)           # 9 KB/part

        xTr = xT[:].rearrange("(t p) n -> t p n", p=P)
        wqk_t = wqkT[:].rearrange("(t p) m -> p t m", p=P)
        wv_t = wvT[:].rearrange("(t p) m -> p t m", p=P)
        wp_t = wpT[:].rearrange("(t p) m -> p t m", p=P)

        # DMA: the gating pieces first, interleaved across both HWDGE
        # queues so the k-outer qk(0) prologue can chase the chunks: even
        # x k-tiles on sync, odd x k-tiles interleaved with the wqk pair-0
        # chunks on scalar.
        nc.sync.dma_start(out=xT_sb[:, 0], in_=xTr[0])
        nc.scalar.dma_start(
            out=wqkT_sb[:, 0:2, 0:256], in_=wqk_t[:, 0:2, 0:256])
        nc.sync.dma_start(out=xT_sb[:, 2], in_=xTr[2])
        nc.scalar.dma_start(out=xT_sb[:, 1], in_=xTr[1])
        nc.sync.dma_start(out=xT_sb[:, 4], in_=xTr[4])
        nc.scalar.dma_start(
            out=wqkT_sb[:, 2:4, 0:256], in_=wqk_t[:, 2:4, 0:256])
        nc.scalar.dma_start(out=xT_sb[:, 3], in_=xTr[3])
        nc.scalar.dma_start(
            out=wqkT_sb[:, 4:6, 0:256], in_=wqk_t[:, 4:6, 0:256])
        nc.scalar.dma_start(out=xT_sb[:, 5], in_=xTr[5])
        for t in range(1, NPAIR):
            nc.scalar.dma_start(
                out=wqkT_sb[:, :, t * 256:(t + 1) * 256],
                in_=wqk_t[:, :, t * 256:(t + 1) * 256],
            )
        for t in range(NPAIR):
            nc.sync.dma_start(
                out=wvT_sb[:, :, t * P:(t + 1) * P],
                in_=wv_t[:, :, t * P:(t + 1) * P],
            )
        nc.vector.memset(v_sb[:, :, :, D], 1.0)
        nc.vector.memset(ones_row, 1.0)
        nc.scalar.dma_start(out=wpT_sb, in_=wp_t)
        nc.sync.dma_start(out=bias_sb, in_=bias[:].to_broadcast((P, DIM)))
        # warm the ScalarE activation table for Exp during the DMA-in phase
        # (the implicit ACT_TABLE_LOAD is 1.3us and otherwise lands in front
        # of the first real exp)
        warm = rp.tile([1, 16], FP, tag="warm")
        nc.scalar.activation(out=warm, in_=ones_row[:, 0:16],
                             func=Exp, scale=1.0)

        # ---- stage-1 PE work generators (filler units; yields are ~us cost
        # estimates used for schedule pacing) ----
        def gen_qk(t):
            """qk pair-tile t -> qkT_sb[:, 2t] (q) and [:, 2t+1] (k)."""
            for which in range(2):
                for lo, hi in _chunks(N, 512):
                    ps = s1ps.tile([P, 512], FP, tag="s1")
                    for k in range(KT):
                        nc.tensor.matmul(
                            ps,
                            wqkT_sb[:, k, t * 256 + which * P:
                                    t * 256 + (which + 1) * P],
                            xT_sb[:, k, lo:hi],
                            start=(k == 0),
                            stop=(k == KT - 1),
                        )
                    nc.vector.tensor_copy(
                        out=qkT_sb[:, 2 * t + which, lo:hi], in_=ps)
                    yield 1.65

        def gen_v(t):
            """v pair-slice t -> v_sb[:, :, 2t:2t+2, 0:D]."""
            for half in range(2):
                ps = s1ps.tile([P, 512], FP, tag="s1")
                for jj in range(4):
                    j = half * 4 + jj
                    for k in range(KT):
                        nc.tensor.matmul(
                            ps[:, jj * P:(jj + 1) * P],
                            xT_sb[:, k, j * P:(j + 1) * P],
                            wvT_sb[:, k, t * P:(t + 1) * P],
                            start=(k == 0),
                            stop=(k == KT - 1),
                        )
                    yield 0.40
                nc.vector.tensor_copy(
                    out=v_sb[:, half * 4:(half + 1) * 4, 2 * t:2 * t + 2, 0:D],
                    in_=ps.rearrange("p (j g d) -> p j g d", g=2, d=D),
                )

        def gen_proj_partial():
            """Output-projection contributions of k-tiles 0..4, SBUF-
            accumulated into y_acc; the last units drain in the epilogue so
            the PE never idles while the final r-chains run on DVE."""
            for i in range(NT):
                for lo, hi in _chunks(DIM, 512):
                    ps = s1ps.tile([P, 512], FP, tag="s1")
                    for k in range(KT - 1):
                        nc.tensor.matmul(
                            ps[:, 0:hi - lo],
                            oT_sb[:, k, i * P:(i + 1) * P],
                            wpT_sb[:, k, lo:hi],
                            start=(k == 0),
                            stop=(k == KT - 2),
                        )
                    nc.vector.tensor_tensor(
                        out=y_acc[:, i, lo:hi], in0=ps[:, 0:hi - lo],
                        in1=bias_sb[:, lo:hi], op=Add,
                    )
                    yield 1.35 if hi - lo == 512 else 0.70

        # ---- attention ----
        ex_store = [[] for _ in range(H)]

        # Global filler work queue: [name, generator]. Pulled cost-weighted
        # so each ST j-step spans enough PE work for the previous step's
        # exps to free the st PSUM slots (else the pair's second ST
        # serializes instead of running row-tile-concurrent).
        work = []

        def pull(budget):
            while budget > 0.0 and work:
                try:
                    budget -= next(work[0][1])
                except StopIteration:
                    work.pop(0)

        def drain(name):
            while any(nm == name for nm, _ in work):
                try:
                    next(work[0][1])
                except StopIteration:
                    work.pop(0)

        # Deferred work cells. pend_exp holds the last-issued ST pair whose
        # exps go out at the TOP of the next cycle (so ST matmuls are issued
        # ~a full cycle after the exps that free their PSUM slots, and never
        # head-of-line-block the PE). pend_rc holds an r-chain tail closure
        # (lbc matmuls + reciprocal + normalize) flushed one unit after its
        # DVE prefix was issued, so the PE reaches the lbc matmuls after the
        # prefix has had time to run.
        pend_exp = [None]
        pend_rc = []

        def flush_exps():
            if pend_exp[0] is None:
                return
            stA, stB, hA, hB = pend_exp[0]
            pend_exp[0] = None
            exA = expp.tile([P, N], MMDT, tag="exp")
            exB = expp.tile([P, N], MMDT, tag="exp")
            nc.scalar.activation(out=exA, in_=stA, func=Exp,
                                 scale=float(SCALE))
            nc.scalar.activation(out=exB, in_=stB, func=Exp,
                                 scale=float(SCALE))
            ex_store[hA].append(exA)
            ex_store[hB].append(exB)

        def flush_rc():
            if pend_rc:
                pend_rc.pop(0)()

        def st_phase(t, budget):
            """ST+exp j-loop for BOTH heads of pair t. The two heads' C=64
            ST matmuls are issued back-to-back with explicit row-tile
            positions (0,0)/(64,0) so they execute concurrently in the PE
            array."""
            for j in range(NT):
                flush_exps()
                pull(budget)
                flush_rc()
                stA = stps.tile([P, N], FP, tag="st")
                stB = stps.tile([P, N], FP, tag="st")
                kslc = slice(j * P, (j + 1) * P)
                for lo, hi in _chunks(N, 512):
                    nc.tensor.matmul(
                        stA[:, lo:hi],
                        qkT_sb[0:D, 2 * t + 1, kslc],
                        qkT_sb[0:D, 2 * t, lo:hi],
                        start=True, stop=True,
                        tile_position=(0, 0),
                    )
                    nc.tensor.matmul(
                        stB[:, lo:hi],
                        qkT_sb[D:P, 2 * t + 1, kslc],
                        qkT_sb[D:P, 2 * t, lo:hi],
                        start=True, stop=True,
                        tile_position=(64, 0),
                    )
                pend_exp[0] = (stA, stB, 2 * t, 2 * t + 1)

        def pv_units(t):
            """PV + r-chain for pair t, as ~1us filler units. PV is chunk-
            phased: for each head, accumulate OT chunk c over all j (the ex
            tiles are all in SBUF by the time these run). Each unit first
            flushes any deferred r-chain tail AFTER issuing its own PE work,
            giving the tail's DVE prefix (the l-row copies) time to run."""
            for h in (2 * t, 2 * t + 1):
                hp = (h % 2) * D
                exs = ex_store[h]
                ota = otps.tile([P, 512], FP, tag="ot")
                otb = otps.tile([P, 512], FP, tag="ot")
                for c, ot in ((0, ota), (1, otb)):
                    for jhalf in range(2):
                        for j in range(jhalf * 4, jhalf * 4 + 4):
                            nc.tensor.matmul(
                                ot[0:D + 1, :],
                                v_sb[:, j, h, :],
                                exs[j][:, c * 512:(c + 1) * 512],
                                start=(j == 0),
                                stop=(j == NT - 1),
                            )
                        flush_rc()
                        yield 1.05
                # r-chain: l rows (bf16) to SBUF now (DVE prefix); the two
                # tail stages (per chunk: ones x l broadcast matmul into an
                # s1-pool PSUM slot -> fp32 approx reciprocal -> normalize
                # fused into the OT evacuation) are deferred to later flush
                # points so the PE reaches each broadcast matmul after its
                # DVE dependencies completed.
                la = rp.tile([1, 512], MMDT, tag="lrowa")
                lb = rp.tile([1, 512], MMDT, tag="lrowb")
                nc.vector.tensor_copy(out=la, in_=ota[D:D + 1, :])
                nc.vector.tensor_copy(out=lb, in_=otb[D:D + 1, :])

                def rc_stage(lrow, ot, lo, hi, hp=hp):
                    def run():
                        lbc = s1ps.tile([P, 512], FP, tag="s1")
                        nc.tensor.matmul(lbc, ones_row, lrow,
                                         start=True, stop=True)
                        rb_sb = rp.tile([P, 512], FP, tag="rb")
                        nc.vector.reciprocal_approx_fast(out=rb_sb, in_=lbc)
                        nc.vector.tensor_tensor(
                            out=oT_sb[hp:hp + D, t, lo:hi], in0=ot[0:D],
                            in1=rb_sb[0:D], op=Mult,
                        )
                    return run

                pend_rc.append(rc_stage(la, ota, 0, 512))
                pend_rc.append(rc_stage(lb, otb, 512, N))
                ex_store[h] = []
                yield 0.40

        # ---- prologue: qk(0) k-outer (in the ST psum slots) so the
        # matmuls chase the per-k-tile x chunk DMAs ----
        stq = stps.tile([P, N], FP, tag="st")
        stk = stps.tile([P, N], FP, tag="st")
        for k in range(KT):
            for which, stt in ((0, stq), (1, stk)):
                for lo, hi in _chunks(N, 512):
                    nc.tensor.matmul(
                        stt[:, lo:hi],
                        wqkT_sb[:, k, which * P:(which + 1) * P],
                        xT_sb[:, k, lo:hi],
                        start=(k == 0),
                        stop=(k == KT - 1),
                    )
        nc.vector.tensor_copy(out=qkT_sb[:, 0], in_=stq)
        nc.vector.tensor_copy(out=qkT_sb[:, 1], in_=stk)

        # ---- pair pipeline ----
        work.append(["v0", gen_v(0)])
        for tt in range(1, NPAIR):
            work.append([f"qk{tt}", gen_qk(tt)])
            work.append([f"v{tt}", gen_v(tt)])
        work.append(["proj", gen_proj_partial()])

        for t in range(NPAIR):
            if t >= 1:
                work.insert(0, [f"pv{t - 1}", pv_units(t - 1)])
            st_phase(t, budget=2.8 if t == NPAIR - 1 else 2.4)
            if t + 1 < NPAIR:
                drain(f"qk{t + 1}")

        # epilogue: interleave pair-5 PV/r-chain with remaining filler so
        # the PE stays dense through the last r-chains (an idle PE here
        # re-throttles HAM and doubles the stage-3 matmul times)
        flush_exps()
        pv5 = pv_units(NPAIR - 1)
        for _ in pv5:
            pull(0.55)
            flush_rc()
        # bridge the final r-chain DVE latency with the remaining proj
        # filler so the PE stays busy until stage 3 can start
        while pend_rc or work:
            flush_rc()
            pull(0.8)

      # -------- stage 3: last projection k-tile (5) + combine --------
      with (
            tc.tile_pool(name="s3y", bufs=4) as s3y,
            tc.tile_pool(name="s3ps", bufs=4, space="PSUM") as s3ps,
      ):
            yr = y[:].rearrange("(i p) e -> i p e", p=P)
            for i in range(NT):
                ps = s3ps.tile([P, DIM], FP, tag="y")
                for lo, hi in _chunks(DIM, 512):
                    nc.tensor.matmul(
                        ps[:, lo:hi],
                        oT_sb[:, KT - 1, i * P:(i + 1) * P],
                        wpT_sb[:, KT - 1, lo:hi],
                        start=True,
                        stop=True,
                    )
                y_sb = s3y.tile([P, DIM], MMDT, tag="ysb")
                nc.vector.tensor_tensor(
                    out=y_sb, in0=ps, in1=y_acc[:, i], op=Add,
                )
                nc.sync.dma_start(out=yr[i], in_=y_sb)


def prep_inputs(x, w_qkv, w_proj, b_proj):
    x = np.asarray(x, dtype=np.float32)
    w_qkv = np.asarray(w_qkv, dtype=np.float32)
    w_proj = np.asarray(w_proj, dtype=np.float32)
    b_proj = np.asarray(b_proj, dtype=np.float32)

    w_r = w_qkv.reshape(H, D, 3, DIM)  # rows ordered (h, d, qkv)
    wq = w_r[:, :, 0, :].reshape(DIM, DIM)  # rows (h, d)
    wk = w_r[:, :, 1, :].reshape(DIM, DIM)
    wv = w_r[:, :, 2, :].reshape(DIM, DIM)
    # pair-blocked qk: columns [q_t (128) | k_t (128)] for t = 0..5
    wqk_pairs = np.empty((2 * DIM, DIM), dtype=np.float32)
    for t in range(NPAIR):
        wqk_pairs[t * 256:t * 256 + P] = wq[t * P:(t + 1) * P]
        wqk_pairs[t * 256 + P:(t + 1) * 256] = wk[t * P:(t + 1) * P]
    wqkT = np.ascontiguousarray(wqk_pairs.T).astype(NP_MMDT)    # [768, 1536]
    wvT = np.ascontiguousarray(wv.T).astype(NP_MMDT)            # [768, 768]
    wpT = np.ascontiguousarray(w_proj.T).astype(NP_MMDT)        # [768, 768]
    xT = np.ascontiguousarray(x.transpose(0, 2, 1)).astype(NP_MMDT)
    bias = np.ascontiguousarray(b_proj.reshape(1, DIM))
    return xT, wqkT, wvT, wpT, bias


_NC = None
last_results = None


def get_nc():
    global _NC
    if _NC is None:
        _NC = build_nc()
    return _NC


def kernel(x, w_qkv, w_proj, b_proj):
    global last_results
    from concourse.bass_utils import run_bass_kernel_spmd

    nc = get_nc()
    xT, wqkT, wvT, wpT, bias = prep_inputs(x, w_qkv, w_proj, b_proj)
    in_maps = [
        {"xT": xT[c], "wqkT": wqkT, "wvT": wvT, "wpT": wpT, "bias": bias}
        for c in range(B)
    ]
    res = run_bass_kernel_spmd(nc, in_maps, core_ids=list(range(B)))
    last_results = res
    return np.stack(
        [np.asarray(res.results[c]["y"], dtype=np.float32) for c in range(B)],
        axis=0,
    )
